# revision 62
# baseline (speedup 1.0000x reference)
"""Trainium2 Bass kernel for CustomPatchEmbedding.

Computes, for each (batch, patch): out[b, n, :] = W @ patch(b, n) + bias where
patch(b, n) is a 16x16x3 window of x[b] centered at centers[b, n].

Strategy (data parallel over 8 NeuronCores, 4 images / 2304 patches per core):
  host: each image is pre-sliced into 369 column slabs (one per possible
        patch start column, 16 px wide, full height), channel-last bf16 with
        rows packed in (even, odd) row-pair order, so that EVERY patch is one
        contiguous 1536-byte run in HBM. One int32 element offset per patch.
  device, per group of up to 512 patches ([128, 384, 512, 512, 512, 256]):
    1. One indirect DMA per 128 patches gathers whole patches (1536B
       descriptor per partition -- real HW supports exactly one gather
       descriptor per partition per indirect DMA) into SBUF token layout
       [128 part = patch%128, rank = patch//128, 768 elems].
    2. ONE SBUF-source dma_gather(transpose=True) per group re-tiles the
       patches into lhsT layout [128 k, 6 k-tiles, gn patches] using the DMA
       xbar -- no PE transposes, no DVE copies.
    3. 36 matmuls (6 e-tiles x 6 k-tiles, N=gn): stationary = W k/e tile,
       streaming = patch k-tile; accumulate [128 e, gn patch] f32 in PSUM.
    4. PSUM drain + per-partition bias + bf16 cast fused into one ACT (or
       DVE, alternating) instruction per e-tile.
    5. DMA the [128 e, 6 et, gn] bf16 result to a transposed DRAM output
       [E, 2304]; host casts/transposes back to [B, N, E] f32.

Self-contained: hardcodes all shapes; host side only shards/reshapes inputs
and builds constant index tables.
"""

import numpy as np
import ml_dtypes

import concourse.bass as bass
import concourse.bacc as bacc_mod
import concourse.mybir as mybir
import concourse.tile as tile
from concourse.bass import IndirectOffsetOnAxis

# Problem shapes (hardcoded per contract).
B, C, H, W = 32, 3, 384, 384
N, E, P = 576, 768, 16
NCORES = 8
BPC = B // NCORES            # images per core = 4
NPATCH = BPC * N             # patches per core = 2304
K = C * P * P                # contraction = 768
KT = K // 128                # k-tiles = 6
ET = E // 128                # e-tiles = 6
RANKS = NPATCH // 128        # 128-patch ranks per core = 18
GROUPS = [128, 384, 512, 512, 512, 256]   # patch groups (sum = NPATCH);
# small first group shortens the gather->first-matmul startup chain, small
# last group shortens the compute tail after the final gather.
NPAIR = P // 2               # row-pairs per patch = 8
ROWB = P * C * 2             # bf16 elems per slab pair-row = 96
# The host pre-slices each image into 369 vertical slabs (one per possible
# patch start column), 16 px wide, pair-packed rows (even pairs (2i,2i+1)
# then odd pairs (2i+1,2i+2)), channel-last. A whole patch is then ONE
# contiguous 1536B run: real HW supports exactly one gather descriptor per
# partition per indirect DMA, so this makes the full patch gather 18
# instructions of 128 descriptors.
PAIRS_E = H // 2             # 192 even pair-rows per slab
PAIRS_O = H // 2 - 1         # 191 odd pair-rows per slab
NSLAB = H - P + 1            # 369 slabs (patch start columns)
SLABE = (PAIRS_E + PAIRS_O) * ROWB   # elems per slab = 36768
XIMG = NSLAB * SLABE         # elems per image in the packed layout

F32 = mybir.dt.float32
BF16 = mybir.dt.bfloat16
I32 = mybir.dt.int32
I16 = mybir.dt.int16

IDENT = mybir.ActivationFunctionType.Identity


def build_program():
    nc = bacc_mod.Bacc()

    x_t = nc.dram_tensor("x", [BPC, XIMG], BF16, kind="ExternalInput")
    # offs[p, t]: element offset of the whole core-patch (t*128+p) block.
    offs_t = nc.dram_tensor("offs", [128, RANKS], I32, kind="ExternalInput")
    # wt[p, et, k, em]: W[k-tile k, k-in-tile p, e = et*128+em], e-tile major
    # so the first matmul only waits for the first 1/6th of the load.
    wt_t = nc.dram_tensor("wt", [128, ET, KT, 128], BF16,
                          kind="ExternalInput")
    bias_t = nc.dram_tensor("bias", [128, ET], F32, kind="ExternalInput")
    gidx_t = nc.dram_tensor("gidx", [128, 32], I16, kind="ExternalInput")
    ident_t = nc.dram_tensor("ident", [128, 128], BF16, kind="ExternalInput")
    out_t = nc.dram_tensor("out", [E, NPATCH], BF16, kind="ExternalOutput")

    # x viewed as [1, Nelem] so gather offsets are element-granular (coef=1).
    x_flat = x_t[:].rearrange("b n -> () (b n)")
    # out viewed as [128 part, ET, NPATCH] for the per-group result writes.
    out_v = out_t[:].rearrange("(et ep) n -> ep et n", ep=128)

    with tile.TileContext(nc) as tc:
        with (
            tc.tile_pool(name="consts", bufs=1) as constp,
            tc.tile_pool(name="lhs", bufs=3) as lhsp,
            tc.tile_pool(name="psum", bufs=6, space="PSUM") as psump,
            tc.tile_pool(name="psumT", bufs=2, space="PSUM") as psumTp,
            tc.tile_pool(name="outp", bufs=3) as outp,
        ):
            # ---- Load constants / replicated weights ----
            offs_sb = constp.tile([128, RANKS], I32, tag="offs")
            nc.sync.dma_start(out=offs_sb[:], in_=offs_t[:])
            wt_sb = constp.tile([128, ET, KT, 128], BF16, tag="wt")
            for et in range(ET):
                nc.sync.dma_start(out=wt_sb[:, et, :, :], in_=wt_t[:, et, :, :])
            bias_sb = constp.tile([128, ET], F32, tag="bias")
            nc.sync.dma_start(out=bias_sb[:], in_=bias_t[:])
            gidx_sb = constp.tile([128, 32], I16, tag="gidx")
            nc.sync.dma_start(out=gidx_sb[:], in_=gidx_t[:])
            ident_sb = constp.tile([128, 128], BF16, tag="ident")
            nc.sync.dma_start(out=ident_sb[:], in_=ident_t[:])

            # 1. Batched row gathers: all gn*16 rows of a group in one SWDGE
            #    op. Issued one group ahead of the consuming transpose so the
            #    in-order Pool sequencer doesn't stall a later gather behind
            #    an earlier transpose's data wait (and the shared DMA engines
            #    aren't hogged by a burst of gathers up-front).
            rstart = [0]
            for gn in GROUPS:
                rstart.append(rstart[-1] + gn // 128)
            praws = {}

            def issue_gather(g):
                gn = GROUPS[g]
                nr = gn // 128
                praw = constp.tile([128, nr, K], BF16, tag=f"praw_{g}")
                for r in range(nr):
                    nc.gpsimd.indirect_dma_start(
                        out=praw[:, r, :],
                        out_offset=None,
                        in_=x_flat,
                        in_offset=IndirectOffsetOnAxis(
                            ap=offs_sb[:, rstart[g] + r:rstart[g] + r + 1],
                            axis=1),
                    )
                praws[g] = praw

            # Warm the PE HAM clock-gate before real work arrives: ~45
            # back-to-back tiny matmuls on a zeroed tile keep the PE busy
            # through its 3.4us activity window, so the first real matmuls
            # run at 2.4 GHz instead of 1.2. Results are never read.
            warm = constp.tile([128, 128], BF16, tag="warm")
            nc.vector.memset(warm[:], 0.0)
            wps = psump.tile([128, 512], F32, tag="ps")
            for _ in range(45):
                nc.tensor.matmul(out=wps[:, 0:64], lhsT=warm[:],
                                 rhs=warm[:, 0:64], start=True, stop=True)

            issue_gather(0)

            r0 = 0
            ng = len(GROUPS)
            for g, gn in enumerate(GROUPS):
                nr = gn // 128
                praw = praws[g]
                for h in range(g + 1, min(g + 2 if g < ng - 3 else g + 3, ng)):
                    if h not in praws:
                        issue_gather(h)

                # 2. Transpose to lhsT [128 k, KT, gn]. Group 0 uses PE
                #    transposes (its data is ready long before the SWDGE DMA
                #    ring drains the prefetch gathers queued ahead of an
                #    xbar transpose); later groups use ONE SBUF-source
                #    dma_gather(transpose=True) each: token t (= local patch)
                #    lives at partition t%128, free stripe t//128 (1536B).
                lhsT = lhsp.tile([128, KT, gn], BF16, tag=f"lhs{nr}")
                if g == 0:
                    for r in range(nr):
                        for k in range(KT):
                            psT = psumTp.tile([128, 128], BF16, tag="psT")
                            nc.tensor.transpose(
                                out=psT[:],
                                in_=praw[:, r, k * 128:(k + 1) * 128],
                                identity=ident_sb[:],
                            )
                            nc.vector.tensor_copy(
                                lhsT[:, k, r * 128:(r + 1) * 128], psT[:])
                else:
                    nc.gpsimd.dma_gather(
                        lhsT[:],
                        praw[:].rearrange("p r e -> p (r e)"),
                        gidx_sb[:, 0:gn // 16],
                        gn,
                        gn,
                        K,
                        transpose=True,
                        sbuf_tokens_per_rank=128,
                        sbuf_free_dim_per_rank=K * 2,
                    )

                # 3/4. Matmuls + fused bias drain per e-tile (alternating
                # ACT / DVE so neither engine is the drain bottleneck).
                ot = outp.tile([128, ET, gn], BF16, tag=f"ot{nr}")
                for et in range(ET):
                    ps = psump.tile([128, 512], F32, tag="ps")
                    for k in range(KT):
                        nc.tensor.matmul(
                            out=ps[:, 0:gn],
                            lhsT=wt_sb[:, et, k, :],
                            rhs=lhsT[:, k, :],
                            start=(k == 0), stop=(k == KT - 1),
                        )
                    if et % 2 == 0:
                        nc.scalar.activation(
                            ot[:, et, :], ps[:, 0:gn], IDENT,
                            bias=bias_sb[:, et:et + 1], scale=1.0,
                        )
                    else:
                        nc.vector.tensor_scalar_add(
                            ot[:, et, :], ps[:, 0:gn], bias_sb[:, et:et + 1],
                        )
                # 5. Store the group's slice of the transposed output, split
                # so writes start before all 6 e-tiles have drained (the last
                # group writes per e-tile to shorten the kernel tail).
                nsplit = 2
                step = ET // nsplit
                for s in range(nsplit):
                    nc.sync.dma_start(
                        out=out_v[:, s * step:(s + 1) * step,
                                  r0 * 128:r0 * 128 + gn],
                        in_=ot[:, s * step:(s + 1) * step, :],
                    )
                r0 += nr

    nc.compile()
    return nc


def prepare_inputs(x, centers, proj_w, proj_b):
    """Shard + marshal the full inputs into per-core input maps."""
    x = np.ascontiguousarray(x, dtype=np.float32)
    centers = np.asarray(centers, dtype=np.int64)

    # Channel-last bf16 image, pair-packed ((c, parity) innermost), then
    # sliced into 369 slabs of 16 px: slab sw holds, for each of 383
    # pair-rows, the 96 elems (16 dw x 3 c x 2 r) of columns [sw, sw+16).
    x_cl = x.transpose(0, 2, 3, 1).astype(ml_dtypes.bfloat16)  # [B, H, W, C]
    xe = x_cl.reshape(B, PAIRS_E, 2, W, C).transpose(0, 1, 3, 4, 2)
    xo = (x_cl[:, 1:-1].reshape(B, PAIRS_O, 2, W, C)
          .transpose(0, 1, 3, 4, 2))
    xp = np.concatenate([xe, xo], axis=1)      # [B, 383, W, C, 2]
    xp = xp.reshape(B, PAIRS_E + PAIRS_O, W, C * 2)
    slabs = np.lib.stride_tricks.sliding_window_view(
        xp, P, axis=2)                         # [B, 383, 369, 6, 16]
    x2 = np.ascontiguousarray(
        slabs.transpose(0, 2, 1, 4, 3)         # [B, 369, 383, 16, 6]
    ).reshape(B, XIMG)

    # Weight: k ordered (pair t, dw, c, row-parity r) with dh = 2t + r, to
    # match the gathered row-pair layout; tiled [128 k-in-tile, KT, E].
    wk = (proj_w.reshape(E, C, NPAIR, 2, P)      # [e, c, t, r, dw]
          .transpose(2, 4, 1, 3, 0)              # [t, dw, c, r, e]
          .reshape(K, E).astype(ml_dtypes.bfloat16))
    wt = np.ascontiguousarray(
        wk.reshape(KT, 128, ET, 128)             # [k, p, et, em]
        .transpose(1, 2, 0, 3))                  # [p, et, k, em]

    # Bias with e on partitions: bias[p, et] = proj_b[et*128 + p].
    bias = np.ascontiguousarray(
        np.asarray(proj_b, dtype=np.float32).reshape(ET, 128).T)

    # Gather-transpose index table: value[p, s] = s*16 + p%16 (token ids in
    # output order, wrapped in 16 partitions).
    p_ = np.arange(128)[:, None]
    s_ = np.arange(32)[None, :]
    gidx = (s_ * 16 + (p_ % 16)).astype(np.int16)

    # Per-patch element offset of its contiguous 768-elem block.
    in_maps = []
    for cidx in range(NCORES):
        cen = centers[cidx * BPC:(cidx + 1) * BPC].reshape(NPATCH, 2)
        b_ = np.arange(NPATCH, dtype=np.int64) // N
        sh = cen[:, 0] - P // 2
        sw = cen[:, 1] - P // 2
        par = sh & 1
        h20 = (sh - par) >> 1
        pp0 = par * PAIRS_E + h20          # first pair-row in the slab
        offs = b_ * XIMG + sw * SLABE + pp0 * ROWB   # [NPATCH]
        # offs table layout [p, t] with core-patch id = t*128 + p.
        offs = offs.reshape(RANKS, 128).T
        in_maps.append({
            "x": np.ascontiguousarray(x2[cidx * BPC:(cidx + 1) * BPC]),
            "offs": np.ascontiguousarray(offs.astype(np.int32)),
            "wt": wt,
            "bias": bias,
            "gidx": gidx,
            "ident": np.eye(128, dtype=ml_dtypes.bfloat16),
        })
    return in_maps


def unmarshal_out(arr):
    """Device output [E, NPATCH] bf16 -> [BPC, N, E] f32."""
    return np.ascontiguousarray(
        np.asarray(arr, dtype=np.float32).T.reshape(BPC, N, E))


_PROGRAM_CACHE = {}


def _get_program():
    key = ()
    if key not in _PROGRAM_CACHE:
        _PROGRAM_CACHE[key] = build_program()
    return _PROGRAM_CACHE[key]


def run_on_hw(inputs, trace=False):
    """Returns (full_output [B, N, E] f32, BassKernelResults)."""
    from concourse.bass_utils import run_bass_kernel_spmd

    nc = _get_program()
    in_maps = prepare_inputs(**inputs)
    res = run_bass_kernel_spmd(
        nc, in_maps, core_ids=list(range(NCORES)), trace=trace,
    )
    outs = [unmarshal_out(r["out"]) for r in res.results]
    full = np.concatenate(outs, axis=0)
    return full, res


def kernel(x, centers, proj_w, proj_b):
    out, _ = run_on_hw(dict(x=x, centers=centers, proj_w=proj_w, proj_b=proj_b))
    return out


# revision 74
# speedup vs baseline: 1.0016x; 1.0016x over previous
"""Trainium2 Bass kernel for CustomPatchEmbedding.

Computes, for each (batch, patch): out[b, n, :] = W @ patch(b, n) + bias where
patch(b, n) is a 16x16x3 window of x[b] centered at centers[b, n].

Strategy (data parallel over 8 NeuronCores, 4 images / 2304 patches per core):
  host: each image is pre-sliced into 369 column slabs (one per possible
        patch start column, 16 px wide, full height), channel-last bf16 with
        rows packed in (even, odd) row-pair order, so that EVERY patch is one
        contiguous 1536-byte run in HBM. One int32 element offset per patch.
  device, per group of up to 512 patches ([128, 384, 512, 512, 512, 256]):
    1. One indirect DMA per 128 patches gathers whole patches (1536B
       descriptor per partition -- real HW supports exactly one gather
       descriptor per partition per indirect DMA) into SBUF token layout
       [128 part = patch%128, rank = patch//128, 768 elems].
    2. ONE SBUF-source dma_gather(transpose=True) per group re-tiles the
       patches into lhsT layout [128 k, 6 k-tiles, gn patches] using the DMA
       xbar -- no PE transposes, no DVE copies.
    3. 36 matmuls (6 e-tiles x 6 k-tiles, N=gn): stationary = W k/e tile,
       streaming = patch k-tile; accumulate [128 e, gn patch] f32 in PSUM.
    4. PSUM drain + per-partition bias + bf16 cast fused into one ACT (or
       DVE, alternating) instruction per e-tile.
    5. DMA the [128 e, 6 et, gn] bf16 result to a transposed DRAM output
       [E, 2304]; host casts/transposes back to [B, N, E] f32.

Self-contained: hardcodes all shapes; host side only shards/reshapes inputs
and builds constant index tables.
"""

import numpy as np
import ml_dtypes

import concourse.bass as bass
import concourse.bacc as bacc_mod
import concourse.mybir as mybir
import concourse.tile as tile
from concourse.bass import IndirectOffsetOnAxis

# Problem shapes (hardcoded per contract).
B, C, H, W = 32, 3, 384, 384
N, E, P = 576, 768, 16
NCORES = 8
BPC = B // NCORES            # images per core = 4
NPATCH = BPC * N             # patches per core = 2304
K = C * P * P                # contraction = 768
KT = K // 128                # k-tiles = 6
ET = E // 128                # e-tiles = 6
RANKS = NPATCH // 128        # 128-patch ranks per core = 18
GROUPS = [128, 384, 512, 512, 512, 256]   # patch groups (sum = NPATCH);
# small first group shortens the gather->first-matmul startup chain, small
# last group shortens the compute tail after the final gather.
NPAIR = P // 2               # row-pairs per patch = 8
ROWB = P * C * 2             # bf16 elems per slab pair-row = 96
# The host pre-slices each image into 369 vertical slabs (one per possible
# patch start column), 16 px wide, pair-packed rows (even pairs (2i,2i+1)
# then odd pairs (2i+1,2i+2)), channel-last. A whole patch is then ONE
# contiguous 1536B run: real HW supports exactly one gather descriptor per
# partition per indirect DMA, so this makes the full patch gather 18
# instructions of 128 descriptors.
PAIRS_E = H // 2             # 192 even pair-rows per slab
PAIRS_O = H // 2 - 1         # 191 odd pair-rows per slab
NSLAB = H - P + 1            # 369 slabs (patch start columns)
SLABE = (PAIRS_E + PAIRS_O) * ROWB   # elems per slab = 36768
XIMG = NSLAB * SLABE         # elems per image in the packed layout

F32 = mybir.dt.float32
BF16 = mybir.dt.bfloat16
I32 = mybir.dt.int32
I16 = mybir.dt.int16

IDENT = mybir.ActivationFunctionType.Identity


def build_program():
    nc = bacc_mod.Bacc()

    x_t = nc.dram_tensor("x", [BPC, XIMG], BF16, kind="ExternalInput")
    # offs[p, t]: element offset of the whole core-patch (t*128+p) block.
    offs_t = nc.dram_tensor("offs", [128, RANKS], I32, kind="ExternalInput")
    # wt[p, et, k, em]: W[k-tile k, k-in-tile p, e = et*128+em], e-tile major
    # so the first matmul only waits for the first 1/6th of the load.
    wt_t = nc.dram_tensor("wt", [128, ET, KT, 128], BF16,
                          kind="ExternalInput")
    bias_t = nc.dram_tensor("bias", [128, ET], F32, kind="ExternalInput")
    gidx_t = nc.dram_tensor("gidx", [128, 32], I16, kind="ExternalInput")
    ident_t = nc.dram_tensor("ident", [128, 128], BF16, kind="ExternalInput")
    out_t = nc.dram_tensor("out", [E, NPATCH], BF16, kind="ExternalOutput")

    # x viewed as [1, Nelem] so gather offsets are element-granular (coef=1).
    x_flat = x_t[:].rearrange("b n -> () (b n)")
    # out viewed as [128 part, ET, NPATCH] for the per-group result writes.
    out_v = out_t[:].rearrange("(et ep) n -> ep et n", ep=128)

    with tile.TileContext(nc) as tc:
        with (
            tc.tile_pool(name="consts", bufs=1) as constp,
            tc.tile_pool(name="lhs", bufs=3) as lhsp,
            tc.tile_pool(name="psum", bufs=6, space="PSUM") as psump,
            tc.tile_pool(name="psumT", bufs=2, space="PSUM") as psumTp,
            tc.tile_pool(name="outp", bufs=3) as outp,
        ):
            # ---- Load constants / replicated weights ----
            offs_sb = constp.tile([128, RANKS], I32, tag="offs")
            nc.sync.dma_start(out=offs_sb[:], in_=offs_t[:])
            wt_sb = constp.tile([128, ET, KT, 128], BF16, tag="wt")
            for et in range(ET):
                nc.sync.dma_start(out=wt_sb[:, et, :, :], in_=wt_t[:, et, :, :])
            bias_sb = constp.tile([128, ET], F32, tag="bias")
            nc.sync.dma_start(out=bias_sb[:], in_=bias_t[:])
            gidx_sb = constp.tile([128, 32], I16, tag="gidx")
            nc.sync.dma_start(out=gidx_sb[:], in_=gidx_t[:])
            ident_sb = constp.tile([128, 128], BF16, tag="ident")
            nc.sync.dma_start(out=ident_sb[:], in_=ident_t[:])

            # 1. Batched row gathers: all gn*16 rows of a group in one SWDGE
            #    op. Issued one group ahead of the consuming transpose so the
            #    in-order Pool sequencer doesn't stall a later gather behind
            #    an earlier transpose's data wait (and the shared DMA engines
            #    aren't hogged by a burst of gathers up-front).
            rstart = [0]
            for gn in GROUPS:
                rstart.append(rstart[-1] + gn // 128)
            praws = {}

            def issue_gather(g):
                gn = GROUPS[g]
                nr = gn // 128
                praw = constp.tile([128, nr, K], BF16, tag=f"praw_{g}")
                for r in range(nr):
                    nc.gpsimd.indirect_dma_start(
                        out=praw[:, r, :],
                        out_offset=None,
                        in_=x_flat,
                        in_offset=IndirectOffsetOnAxis(
                            ap=offs_sb[:, rstart[g] + r:rstart[g] + r + 1],
                            axis=1),
                    )
                praws[g] = praw

            # Warm the PE HAM clock-gate before real work arrives: ~45
            # back-to-back tiny matmuls on a zeroed tile keep the PE busy
            # through its 3.4us activity window, so the first real matmuls
            # run at 2.4 GHz instead of 1.2. Results are never read.
            warm = constp.tile([128, 128], BF16, tag="warm")
            nc.vector.memset(warm[:], 0.0)
            wps = psump.tile([128, 512], F32, tag="ps")
            for _ in range(45):
                nc.tensor.matmul(out=wps[:, 0:64], lhsT=warm[:],
                                 rhs=warm[:, 0:64], start=True, stop=True)

            issue_gather(0)

            r0 = 0
            ng = len(GROUPS)
            for g, gn in enumerate(GROUPS):
                nr = gn // 128
                praw = praws[g]
                for h in range(g + 1, min(g + 2 if g < ng - 3 else g + 3, ng)):
                    if h not in praws:
                        issue_gather(h)

                # 2. Transpose to lhsT [128 k, KT, gn]. Group 0 uses PE
                #    transposes (its data is ready long before the SWDGE DMA
                #    ring drains the prefetch gathers queued ahead of an
                #    xbar transpose); later groups use ONE SBUF-source
                #    dma_gather(transpose=True) each: token t (= local patch)
                #    lives at partition t%128, free stripe t//128 (1536B).
                lhsT = lhsp.tile([128, KT, gn], BF16, tag=f"lhs{nr}")
                if g == 0:
                    for r in range(nr):
                        for k in range(KT):
                            psT = psumTp.tile([128, 128], BF16, tag="psT")
                            nc.tensor.transpose(
                                out=psT[:],
                                in_=praw[:, r, k * 128:(k + 1) * 128],
                                identity=ident_sb[:],
                            )
                            nc.vector.tensor_copy(
                                lhsT[:, k, r * 128:(r + 1) * 128], psT[:])
                else:
                    nc.gpsimd.dma_gather(
                        lhsT[:],
                        praw[:].rearrange("p r e -> p (r e)"),
                        gidx_sb[:, 0:gn // 16],
                        gn,
                        gn,
                        K,
                        transpose=True,
                        sbuf_tokens_per_rank=128,
                        sbuf_free_dim_per_rank=K * 2,
                    )

                # 3/4. Matmuls + fused bias drain per e-tile (alternating
                # ACT / DVE so neither engine is the drain bottleneck).
                ot = outp.tile([128, ET, gn], BF16, tag=f"ot{nr}")
                for et in range(ET):
                    ps = psump.tile([128, 512], F32, tag="ps")
                    for k in range(KT):
                        nc.tensor.matmul(
                            out=ps[:, 0:gn],
                            lhsT=wt_sb[:, et, k, :],
                            rhs=lhsT[:, k, :],
                            start=(k == 0), stop=(k == KT - 1),
                        )
                    if et % 2 == 0:
                        nc.scalar.activation(
                            ot[:, et, :], ps[:, 0:gn], IDENT,
                            bias=bias_sb[:, et:et + 1], scale=1.0,
                        )
                    else:
                        nc.vector.tensor_scalar_add(
                            ot[:, et, :], ps[:, 0:gn], bias_sb[:, et:et + 1],
                        )
                # 5. Store the group's slice of the transposed output, split
                # so writes start before all 6 e-tiles have drained (the last
                # group writes per e-tile to shorten the kernel tail).
                nsplit = 3
                step = ET // nsplit
                for s in range(nsplit):
                    nc.sync.dma_start(
                        out=out_v[:, s * step:(s + 1) * step,
                                  r0 * 128:r0 * 128 + gn],
                        in_=ot[:, s * step:(s + 1) * step, :],
                    )
                r0 += nr

    nc.compile()
    return nc


def prepare_inputs(x, centers, proj_w, proj_b):
    """Shard + marshal the full inputs into per-core input maps."""
    x = np.ascontiguousarray(x, dtype=np.float32)
    centers = np.asarray(centers, dtype=np.int64)

    # Channel-last bf16 image, pair-packed ((c, parity) innermost), then
    # sliced into 369 slabs of 16 px: slab sw holds, for each of 383
    # pair-rows, the 96 elems (16 dw x 3 c x 2 r) of columns [sw, sw+16).
    x_cl = x.transpose(0, 2, 3, 1).astype(ml_dtypes.bfloat16)  # [B, H, W, C]
    xe = x_cl.reshape(B, PAIRS_E, 2, W, C).transpose(0, 1, 3, 4, 2)
    xo = (x_cl[:, 1:-1].reshape(B, PAIRS_O, 2, W, C)
          .transpose(0, 1, 3, 4, 2))
    xp = np.concatenate([xe, xo], axis=1)      # [B, 383, W, C, 2]
    xp = xp.reshape(B, PAIRS_E + PAIRS_O, W, C * 2)
    slabs = np.lib.stride_tricks.sliding_window_view(
        xp, P, axis=2)                         # [B, 383, 369, 6, 16]
    x2 = np.ascontiguousarray(
        slabs.transpose(0, 2, 1, 4, 3)         # [B, 369, 383, 16, 6]
    ).reshape(B, XIMG)

    # Weight: k ordered (pair t, dw, c, row-parity r) with dh = 2t + r, to
    # match the gathered row-pair layout; tiled [128 k-in-tile, KT, E].
    wk = (proj_w.reshape(E, C, NPAIR, 2, P)      # [e, c, t, r, dw]
          .transpose(2, 4, 1, 3, 0)              # [t, dw, c, r, e]
          .reshape(K, E).astype(ml_dtypes.bfloat16))
    wt = np.ascontiguousarray(
        wk.reshape(KT, 128, ET, 128)             # [k, p, et, em]
        .transpose(1, 2, 0, 3))                  # [p, et, k, em]

    # Bias with e on partitions: bias[p, et] = proj_b[et*128 + p].
    bias = np.ascontiguousarray(
        np.asarray(proj_b, dtype=np.float32).reshape(ET, 128).T)

    # Gather-transpose index table: value[p, s] = s*16 + p%16 (token ids in
    # output order, wrapped in 16 partitions).
    p_ = np.arange(128)[:, None]
    s_ = np.arange(32)[None, :]
    gidx = (s_ * 16 + (p_ % 16)).astype(np.int16)

    # Per-patch element offset of its contiguous 768-elem block.
    in_maps = []
    for cidx in range(NCORES):
        cen = centers[cidx * BPC:(cidx + 1) * BPC].reshape(NPATCH, 2)
        b_ = np.arange(NPATCH, dtype=np.int64) // N
        sh = cen[:, 0] - P // 2
        sw = cen[:, 1] - P // 2
        par = sh & 1
        h20 = (sh - par) >> 1
        pp0 = par * PAIRS_E + h20          # first pair-row in the slab
        offs = b_ * XIMG + sw * SLABE + pp0 * ROWB   # [NPATCH]
        # offs table layout [p, t] with core-patch id = t*128 + p.
        offs = offs.reshape(RANKS, 128).T
        in_maps.append({
            "x": np.ascontiguousarray(x2[cidx * BPC:(cidx + 1) * BPC]),
            "offs": np.ascontiguousarray(offs.astype(np.int32)),
            "wt": wt,
            "bias": bias,
            "gidx": gidx,
            "ident": np.eye(128, dtype=ml_dtypes.bfloat16),
        })
    return in_maps


def unmarshal_out(arr):
    """Device output [E, NPATCH] bf16 -> [BPC, N, E] f32."""
    return np.ascontiguousarray(
        np.asarray(arr, dtype=np.float32).T.reshape(BPC, N, E))


_PROGRAM_CACHE = {}


def _get_program():
    key = ()
    if key not in _PROGRAM_CACHE:
        _PROGRAM_CACHE[key] = build_program()
    return _PROGRAM_CACHE[key]


def run_on_hw(inputs, trace=False):
    """Returns (full_output [B, N, E] f32, BassKernelResults)."""
    from concourse.bass_utils import run_bass_kernel_spmd

    nc = _get_program()
    in_maps = prepare_inputs(**inputs)
    res = run_bass_kernel_spmd(
        nc, in_maps, core_ids=list(range(NCORES)), trace=trace,
    )
    outs = [unmarshal_out(r["out"]) for r in res.results]
    full = np.concatenate(outs, axis=0)
    return full, res


def kernel(x, centers, proj_w, proj_b):
    out, _ = run_on_hw(dict(x=x, centers=centers, proj_w=proj_w, proj_b=proj_b))
    return out


# revision 78
# speedup vs baseline: 1.0036x; 1.0020x over previous
"""Trainium2 Bass kernel for CustomPatchEmbedding.

Computes, for each (batch, patch): out[b, n, :] = W @ patch(b, n) + bias where
patch(b, n) is a 16x16x3 window of x[b] centered at centers[b, n].

Strategy (data parallel over 8 NeuronCores, 4 images / 2304 patches per core):
  host: each image is pre-sliced into 369 column slabs (one per possible
        patch start column, 16 px wide, full height), channel-last bf16 with
        rows packed in (even, odd) row-pair order, so that EVERY patch is one
        contiguous 1536-byte run in HBM. One int32 element offset per patch.
  device, per group of up to 512 patches ([128, 384, 512, 512, 512, 256]):
    1. One indirect DMA per 128 patches gathers whole patches (1536B
       descriptor per partition -- real HW supports exactly one gather
       descriptor per partition per indirect DMA) into SBUF token layout
       [128 part = patch%128, rank = patch//128, 768 elems].
    2. ONE SBUF-source dma_gather(transpose=True) per group re-tiles the
       patches into lhsT layout [128 k, 6 k-tiles, gn patches] using the DMA
       xbar -- no PE transposes, no DVE copies.
    3. 36 matmuls (6 e-tiles x 6 k-tiles, N=gn): stationary = W k/e tile,
       streaming = patch k-tile; accumulate [128 e, gn patch] f32 in PSUM.
    4. PSUM drain + per-partition bias + bf16 cast fused into one ACT (or
       DVE, alternating) instruction per e-tile.
    5. DMA the [128 e, 6 et, gn] bf16 result to a transposed DRAM output
       [E, 2304]; host casts/transposes back to [B, N, E] f32.

Self-contained: hardcodes all shapes; host side only shards/reshapes inputs
and builds constant index tables.
"""

import numpy as np
import ml_dtypes

import concourse.bass as bass
import concourse.bacc as bacc_mod
import concourse.mybir as mybir
import concourse.tile as tile
from concourse.bass import IndirectOffsetOnAxis

# Problem shapes (hardcoded per contract).
B, C, H, W = 32, 3, 384, 384
N, E, P = 576, 768, 16
NCORES = 8
BPC = B // NCORES            # images per core = 4
NPATCH = BPC * N             # patches per core = 2304
K = C * P * P                # contraction = 768
KT = K // 128                # k-tiles = 6
ET = E // 128                # e-tiles = 6
RANKS = NPATCH // 128        # 128-patch ranks per core = 18
GROUPS = [128, 384, 512, 512, 512, 256]   # patch groups (sum = NPATCH);
# small first group shortens the gather->first-matmul startup chain, small
# last group shortens the compute tail after the final gather.
NPAIR = P // 2               # row-pairs per patch = 8
ROWB = P * C * 2             # bf16 elems per slab pair-row = 96
# The host pre-slices each image into 369 vertical slabs (one per possible
# patch start column), 16 px wide, pair-packed rows (even pairs (2i,2i+1)
# then odd pairs (2i+1,2i+2)), channel-last. A whole patch is then ONE
# contiguous 1536B run: real HW supports exactly one gather descriptor per
# partition per indirect DMA, so this makes the full patch gather 18
# instructions of 128 descriptors.
PAIRS_E = H // 2             # 192 even pair-rows per slab
PAIRS_O = H // 2 - 1         # 191 odd pair-rows per slab
NSLAB = H - P + 1            # 369 slabs (patch start columns)
SLABE = (PAIRS_E + PAIRS_O) * ROWB   # elems per slab = 36768
XIMG = NSLAB * SLABE         # elems per image in the packed layout

F32 = mybir.dt.float32
BF16 = mybir.dt.bfloat16
I32 = mybir.dt.int32
I16 = mybir.dt.int16

IDENT = mybir.ActivationFunctionType.Identity


def build_program():
    nc = bacc_mod.Bacc()

    x_t = nc.dram_tensor("x", [BPC, XIMG], BF16, kind="ExternalInput")
    # offs[p, t]: element offset of the whole core-patch (t*128+p) block.
    offs_t = nc.dram_tensor("offs", [128, RANKS], I32, kind="ExternalInput")
    # wt[p, et, k, em]: W[k-tile k, k-in-tile p, e = et*128+em], e-tile major
    # so the first matmul only waits for the first 1/6th of the load.
    wt_t = nc.dram_tensor("wt", [128, ET, KT, 128], BF16,
                          kind="ExternalInput")
    bias_t = nc.dram_tensor("bias", [128, ET], F32, kind="ExternalInput")
    gidx_t = nc.dram_tensor("gidx", [128, 32], I16, kind="ExternalInput")
    ident_t = nc.dram_tensor("ident", [128, 128], BF16, kind="ExternalInput")
    out_t = nc.dram_tensor("out", [E, NPATCH], BF16, kind="ExternalOutput")

    # x viewed as [1, Nelem] so gather offsets are element-granular (coef=1).
    x_flat = x_t[:].rearrange("b n -> () (b n)")
    # out viewed as [128 part, ET, NPATCH] for the per-group result writes.
    out_v = out_t[:].rearrange("(et ep) n -> ep et n", ep=128)

    with tile.TileContext(nc) as tc:
        with (
            tc.tile_pool(name="consts", bufs=1) as constp,
            tc.tile_pool(name="lhs", bufs=3) as lhsp,
            tc.tile_pool(name="psum", bufs=6, space="PSUM") as psump,
            tc.tile_pool(name="psumT", bufs=2, space="PSUM") as psumTp,
            tc.tile_pool(name="outp", bufs=3) as outp,
        ):
            # ---- Load constants / replicated weights ----
            offs_sb = constp.tile([128, RANKS], I32, tag="offs")
            nc.sync.dma_start(out=offs_sb[:], in_=offs_t[:])
            wt_sb = constp.tile([128, ET, KT, 128], BF16, tag="wt")
            for et in range(ET):
                nc.sync.dma_start(out=wt_sb[:, et, :, :], in_=wt_t[:, et, :, :])
            bias_sb = constp.tile([128, ET], F32, tag="bias")
            nc.sync.dma_start(out=bias_sb[:], in_=bias_t[:])
            gidx_sb = constp.tile([128, 32], I16, tag="gidx")
            nc.sync.dma_start(out=gidx_sb[:], in_=gidx_t[:])
            ident_sb = constp.tile([128, 128], BF16, tag="ident")
            nc.sync.dma_start(out=ident_sb[:], in_=ident_t[:])

            # 1. Batched row gathers: all gn*16 rows of a group in one SWDGE
            #    op. Issued one group ahead of the consuming transpose so the
            #    in-order Pool sequencer doesn't stall a later gather behind
            #    an earlier transpose's data wait (and the shared DMA engines
            #    aren't hogged by a burst of gathers up-front).
            rstart = [0]
            for gn in GROUPS:
                rstart.append(rstart[-1] + gn // 128)
            praws = {}

            def issue_gather(g):
                gn = GROUPS[g]
                nr = gn // 128
                praw = constp.tile([128, nr, K], BF16, tag=f"praw_{g}")
                for r in range(nr):
                    nc.gpsimd.indirect_dma_start(
                        out=praw[:, r, :],
                        out_offset=None,
                        in_=x_flat,
                        in_offset=IndirectOffsetOnAxis(
                            ap=offs_sb[:, rstart[g] + r:rstart[g] + r + 1],
                            axis=1),
                    )
                praws[g] = praw

            # Warm the PE HAM clock-gate before real work arrives: ~45
            # back-to-back tiny matmuls on a zeroed tile keep the PE busy
            # through its 3.4us activity window, so the first real matmuls
            # run at 2.4 GHz instead of 1.2. Results are never read.
            warm = constp.tile([128, 128], BF16, tag="warm")
            nc.vector.memset(warm[:], 0.0)
            wps = psump.tile([128, 512], F32, tag="ps")
            for _ in range(45):
                nc.tensor.matmul(out=wps[:, 0:64], lhsT=warm[:],
                                 rhs=warm[:, 0:64], start=True, stop=True)

            issue_gather(0)

            r0 = 0
            ng = len(GROUPS)
            for g, gn in enumerate(GROUPS):
                nr = gn // 128
                praw = praws[g]

                # 2. Transpose to lhsT [128 k, KT, gn]. Group 0 uses PE
                #    transposes (its data is ready long before the SWDGE DMA
                #    ring drains the prefetch gathers queued ahead of an
                #    xbar transpose); later groups use ONE SBUF-source
                #    dma_gather(transpose=True) each: token t (= local patch)
                #    lives at partition t%128, free stripe t//128 (1536B).
                lhsT = lhsp.tile([128, KT, gn], BF16, tag=f"lhs{nr}")
                if g == 0:
                    for r in range(nr):
                        for k in range(KT):
                            psT = psumTp.tile([128, 128], BF16, tag="psT")
                            nc.tensor.transpose(
                                out=psT[:],
                                in_=praw[:, r, k * 128:(k + 1) * 128],
                                identity=ident_sb[:],
                            )
                            nc.vector.tensor_copy(
                                lhsT[:, k, r * 128:(r + 1) * 128], psT[:])
                else:
                    nc.gpsimd.dma_gather(
                        lhsT[:],
                        praw[:].rearrange("p r e -> p (r e)"),
                        gidx_sb[:, 0:gn // 16],
                        gn,
                        gn,
                        K,
                        transpose=True,
                        sbuf_tokens_per_rank=128,
                        sbuf_free_dim_per_rank=K * 2,
                    )

                for h in range(g + 1, min(g + 2 if g < ng - 3 else g + 3, ng)):
                    if h not in praws:
                        issue_gather(h)

                # 3/4. Matmuls + fused bias drain per e-tile (alternating
                # ACT / DVE so neither engine is the drain bottleneck).
                ot = outp.tile([128, ET, gn], BF16, tag=f"ot{nr}")
                for et in range(ET):
                    ps = psump.tile([128, 512], F32, tag="ps")
                    for k in range(KT):
                        nc.tensor.matmul(
                            out=ps[:, 0:gn],
                            lhsT=wt_sb[:, et, k, :],
                            rhs=lhsT[:, k, :],
                            start=(k == 0), stop=(k == KT - 1),
                        )
                    if et % 2 == 0:
                        nc.scalar.activation(
                            ot[:, et, :], ps[:, 0:gn], IDENT,
                            bias=bias_sb[:, et:et + 1], scale=1.0,
                        )
                    else:
                        nc.vector.tensor_scalar_add(
                            ot[:, et, :], ps[:, 0:gn], bias_sb[:, et:et + 1],
                        )
                # 5. Store the group's slice of the transposed output, split
                # so writes start before all 6 e-tiles have drained (the last
                # group writes per e-tile to shorten the kernel tail).
                nsplit = 3
                step = ET // nsplit
                for s in range(nsplit):
                    nc.sync.dma_start(
                        out=out_v[:, s * step:(s + 1) * step,
                                  r0 * 128:r0 * 128 + gn],
                        in_=ot[:, s * step:(s + 1) * step, :],
                    )
                r0 += nr

    nc.compile()
    return nc


def prepare_inputs(x, centers, proj_w, proj_b):
    """Shard + marshal the full inputs into per-core input maps."""
    x = np.ascontiguousarray(x, dtype=np.float32)
    centers = np.asarray(centers, dtype=np.int64)

    # Channel-last bf16 image, pair-packed ((c, parity) innermost), then
    # sliced into 369 slabs of 16 px: slab sw holds, for each of 383
    # pair-rows, the 96 elems (16 dw x 3 c x 2 r) of columns [sw, sw+16).
    x_cl = x.transpose(0, 2, 3, 1).astype(ml_dtypes.bfloat16)  # [B, H, W, C]
    xe = x_cl.reshape(B, PAIRS_E, 2, W, C).transpose(0, 1, 3, 4, 2)
    xo = (x_cl[:, 1:-1].reshape(B, PAIRS_O, 2, W, C)
          .transpose(0, 1, 3, 4, 2))
    xp = np.concatenate([xe, xo], axis=1)      # [B, 383, W, C, 2]
    xp = xp.reshape(B, PAIRS_E + PAIRS_O, W, C * 2)
    slabs = np.lib.stride_tricks.sliding_window_view(
        xp, P, axis=2)                         # [B, 383, 369, 6, 16]
    x2 = np.ascontiguousarray(
        slabs.transpose(0, 2, 1, 4, 3)         # [B, 369, 383, 16, 6]
    ).reshape(B, XIMG)

    # Weight: k ordered (pair t, dw, c, row-parity r) with dh = 2t + r, to
    # match the gathered row-pair layout; tiled [128 k-in-tile, KT, E].
    wk = (proj_w.reshape(E, C, NPAIR, 2, P)      # [e, c, t, r, dw]
          .transpose(2, 4, 1, 3, 0)              # [t, dw, c, r, e]
          .reshape(K, E).astype(ml_dtypes.bfloat16))
    wt = np.ascontiguousarray(
        wk.reshape(KT, 128, ET, 128)             # [k, p, et, em]
        .transpose(1, 2, 0, 3))                  # [p, et, k, em]

    # Bias with e on partitions: bias[p, et] = proj_b[et*128 + p].
    bias = np.ascontiguousarray(
        np.asarray(proj_b, dtype=np.float32).reshape(ET, 128).T)

    # Gather-transpose index table: value[p, s] = s*16 + p%16 (token ids in
    # output order, wrapped in 16 partitions).
    p_ = np.arange(128)[:, None]
    s_ = np.arange(32)[None, :]
    gidx = (s_ * 16 + (p_ % 16)).astype(np.int16)

    # Per-patch element offset of its contiguous 768-elem block.
    in_maps = []
    for cidx in range(NCORES):
        cen = centers[cidx * BPC:(cidx + 1) * BPC].reshape(NPATCH, 2)
        b_ = np.arange(NPATCH, dtype=np.int64) // N
        sh = cen[:, 0] - P // 2
        sw = cen[:, 1] - P // 2
        par = sh & 1
        h20 = (sh - par) >> 1
        pp0 = par * PAIRS_E + h20          # first pair-row in the slab
        offs = b_ * XIMG + sw * SLABE + pp0 * ROWB   # [NPATCH]
        # offs table layout [p, t] with core-patch id = t*128 + p.
        offs = offs.reshape(RANKS, 128).T
        in_maps.append({
            "x": np.ascontiguousarray(x2[cidx * BPC:(cidx + 1) * BPC]),
            "offs": np.ascontiguousarray(offs.astype(np.int32)),
            "wt": wt,
            "bias": bias,
            "gidx": gidx,
            "ident": np.eye(128, dtype=ml_dtypes.bfloat16),
        })
    return in_maps


def unmarshal_out(arr):
    """Device output [E, NPATCH] bf16 -> [BPC, N, E] f32."""
    return np.ascontiguousarray(
        np.asarray(arr, dtype=np.float32).T.reshape(BPC, N, E))


_PROGRAM_CACHE = {}


def _get_program():
    key = ()
    if key not in _PROGRAM_CACHE:
        _PROGRAM_CACHE[key] = build_program()
    return _PROGRAM_CACHE[key]


def run_on_hw(inputs, trace=False):
    """Returns (full_output [B, N, E] f32, BassKernelResults)."""
    from concourse.bass_utils import run_bass_kernel_spmd

    nc = _get_program()
    in_maps = prepare_inputs(**inputs)
    res = run_bass_kernel_spmd(
        nc, in_maps, core_ids=list(range(NCORES)), trace=trace,
    )
    outs = [unmarshal_out(r["out"]) for r in res.results]
    full = np.concatenate(outs, axis=0)
    return full, res


def kernel(x, centers, proj_w, proj_b):
    out, _ = run_on_hw(dict(x=x, centers=centers, proj_w=proj_w, proj_b=proj_b))
    return out
